# revision 4
# baseline (speedup 1.0000x reference)
"""Trainium2 Bass kernel for the LogicLayer (difflogic) problem.

out[i, o] = c0[o] + ca[o]*a + cb[o]*b + cab[o]*a*b
  with a = x[i, idx_a[o]], b = x[i, idx_b[o]],
  [c0, ca, cb, cab] = softmax(weights[o]) @ GATE_COEFFS.

Strategy (8 cores, batch-sharded, 512 rows/core), batch-major:
  - x shard resident in SBUF as 4 blocks of [128, 8192].
  - gpsimd.ap_gather pulls a = x[:, idx_a-chunk], b = x[:, idx_b-chunk]
    along the free axis (indices identical for every partition).
  - combine with per-output-column coefficient tensors (pre-broadcast
    across partitions on the host, streamed from HBM per chunk):
      q = (a*cab + cb) * b;  r = a*ca + c0;  out = q + r
    6 DVE tensor_tensor ops, 2 scratch tiles (in-place updates).
"""

import numpy as np

BATCH, IN_DIM, OUT_DIM = 4096, 8192, 8192
N_CORES = 8
ROWS = BATCH // N_CORES  # 512 rows per core
P = 128
N_BB = ROWS // P         # 4 batch blocks per core
OCHUNK = 1024            # output columns per chunk
N_CHUNK = OUT_DIM // OCHUNK  # 8

GATE_COEFFS = np.array([
    [0, 0, 0, 0], [0, 0, 0, 1], [0, 1, 0, -1], [0, 1, 0, 0],
    [0, 0, 1, -1], [0, 0, 1, 0], [0, 1, 1, -2], [0, 1, 1, -1],
    [1, -1, -1, 1], [1, -1, -1, 2], [1, 0, -1, 0], [1, 0, -1, 1],
    [1, -1, 0, 0], [1, -1, 0, 1], [1, 0, 0, -1], [1, 0, 0, 0],
], dtype=np.float32)  # [16, 4]

_CACHE = {}


def _build_nc(n_reps=1):
    import concourse.bacc as bacc
    import concourse.mybir as mybir
    from concourse.tile import TileContext

    f32 = mybir.dt.float32
    i16 = mybir.dt.int16
    Alu = mybir.AluOpType

    nc = bacc.Bacc("TRN2", target_bir_lowering=False, debug=False,
                   num_devices=N_CORES)
    x = nc.dram_tensor("x", [ROWS, IN_DIM], f32, kind="ExternalInput").ap()
    idxw = nc.dram_tensor("idxw", [P, OUT_DIM // 8], i16,
                          kind="ExternalInput").ap()
    cbt = nc.dram_tensor("cbt", [P, 4, OUT_DIM], f32,
                         kind="ExternalInput").ap()
    y = nc.dram_tensor("y", [ROWS, OUT_DIM], f32, kind="ExternalOutput").ap()

    x_t = x.rearrange("(bb p) m -> bb p m", p=P)      # [4, 128, 8192]
    y_t = y.rearrange("(bb p) m -> bb p m", p=P)      # [4, 128, 8192]
    icols = OCHUNK // 16  # idx columns per chunk (64)

    with TileContext(nc) as tc:
        with tc.tile_pool(name="xr", bufs=1) as xpool, \
             tc.tile_pool(name="const", bufs=1) as cpool:
            xa = xpool.tile([P, N_BB, IN_DIM], f32, tag="xa")
            for bb in range(N_BB):
                nc.sync.dma_start(out=xa[:, bb, :], in_=x_t[bb])
            idx_sb = cpool.tile([P, OUT_DIM // 8], i16, tag="idx")
            nc.sync.dma_start(out=idx_sb[:], in_=idxw)

            for rep in range(n_reps):
                with tc.tile_pool(name=f"co{rep}", bufs=2) as copool, \
                     tc.tile_pool(name=f"ab{rep}", bufs=2) as abpool, \
                     tc.tile_pool(name=f"qr{rep}", bufs=2) as qrpool:
                    for c in range(N_CHUNK):
                        cc = copool.tile([P, 4, OCHUNK], f32, tag="cc")
                        nc.sync.dma_start(
                            out=cc[:],
                            in_=cbt[:, :, c * OCHUNK:(c + 1) * OCHUNK])
                        ia = idx_sb[:, c * icols:(c + 1) * icols]
                        ib = idx_sb[:, OUT_DIM // 16 + c * icols:
                                    OUT_DIM // 16 + (c + 1) * icols]
                        c0 = cc[:, 0, :]
                        ca = cc[:, 1, :]
                        cb = cc[:, 2, :]
                        cab = cc[:, 3, :]
                        for bb in range(N_BB):
                            a = abpool.tile([P, OCHUNK], f32, tag="a")
                            nc.gpsimd.ap_gather(
                                out_ap=a[:], in_ap=xa[:, bb, :], idxs_ap=ia,
                                channels=P, num_elems=IN_DIM, d=1,
                                num_idxs=OCHUNK)
                            b = abpool.tile([P, OCHUNK], f32, tag="b")
                            nc.gpsimd.ap_gather(
                                out_ap=b[:], in_ap=xa[:, bb, :], idxs_ap=ib,
                                channels=P, num_elems=IN_DIM, d=1,
                                num_idxs=OCHUNK)
                            q = qrpool.tile([P, OCHUNK], f32, tag="q")
                            r = qrpool.tile([P, OCHUNK], f32, tag="r")
                            # q = (a*cab + cb) * b
                            nc.vector.tensor_mul(q[:], a[:], cab)
                            nc.vector.tensor_add(q[:], q[:], cb)
                            nc.vector.tensor_mul(q[:], q[:], b[:])
                            # r = a*ca + c0
                            nc.vector.tensor_mul(r[:], a[:], ca)
                            nc.vector.tensor_add(r[:], r[:], c0)
                            # out = q + r
                            nc.vector.tensor_add(q[:], q[:], r[:])
                            nc.sync.dma_start(
                                out=y_t[bb][:, c * OCHUNK:(c + 1) * OCHUNK],
                                in_=q[:])
    nc.compile()
    return nc


def _prep_host(x, weights, idx_a, idx_b):
    x = np.ascontiguousarray(np.asarray(x, dtype=np.float32))
    w = np.asarray(weights, dtype=np.float32)
    e = np.exp(w - w.max(axis=1, keepdims=True))
    sm = e / e.sum(axis=1, keepdims=True)
    coeffs = (sm @ GATE_COEFFS).astype(np.float32)          # [8192, 4]
    cbt = np.ascontiguousarray(
        np.broadcast_to(coeffs.T[None, :, :], (P, 4, OUT_DIM))
    ).astype(np.float32)                                     # [128, 4, 8192]
    ia = np.asarray(idx_a).astype(np.int16)
    ib = np.asarray(idx_b).astype(np.int16)

    def wrap(seq):  # j = s*16 + p16 -> [16, len/16] -> tile to 128 partitions
        m = seq.reshape(len(seq) // 16, 16).T
        return np.tile(m, (P // 16, 1))

    idxw = np.ascontiguousarray(
        np.concatenate([wrap(ia), wrap(ib)], axis=1))        # [128, 1024]
    return x, idxw, cbt


def _in_maps(x, weights, idx_a, idx_b):
    xf, idxw, cbt = _prep_host(x, weights, idx_a, idx_b)
    return [{"x": xf[c * ROWS:(c + 1) * ROWS], "idxw": idxw, "cbt": cbt}
            for c in range(N_CORES)]


def kernel(x, weights, idx_a, idx_b):
    from concourse.bass_utils import run_bass_kernel_spmd

    in_maps = _in_maps(x, weights, idx_a, idx_b)
    if "nc" not in _CACHE:
        _CACHE["nc"] = _build_nc()
    nc = _CACHE["nc"]
    res = run_bass_kernel_spmd(nc, in_maps, list(range(N_CORES)))
    out = np.concatenate([res.results[c]["y"] for c in range(N_CORES)], axis=0)
    return out.astype(np.float32)


# revision 5
# speedup vs baseline: 2.6197x; 2.6197x over previous
"""Trainium2 Bass kernel for the LogicLayer (difflogic) problem.

out[i, o] = c0[o] + ca[o]*a + cb[o]*b + cab[o]*a*b
  with a = x[i, idx_a[o]], b = x[i, idx_b[o]],
  [c0, ca, cb, cab] = softmax(weights[o]) @ GATE_COEFFS.

Strategy (8 cores, batch-sharded, 512 rows/core), batch-major:
  - x shard resident in SBUF as 4 blocks of [128, 8192].
  - gpsimd.ap_gather pulls a = x[:, idx_a-chunk], b = x[:, idx_b-chunk]
    along the free axis (indices identical for every partition).
  - combine with per-output-column coefficient tensors (pre-broadcast
    across partitions on the host, streamed from HBM per chunk):
      q = (a*cab + cb) * b;  r = a*ca + c0;  out = q + r
    6 DVE tensor_tensor ops, 2 scratch tiles (in-place updates).
"""

import numpy as np

BATCH, IN_DIM, OUT_DIM = 4096, 8192, 8192
N_CORES = 8
ROWS = BATCH // N_CORES  # 512 rows per core
P = 128
N_BB = ROWS // P         # 4 batch blocks per core
OCHUNK = 512             # output columns per chunk
N_CHUNK = OUT_DIM // OCHUNK  # 8

GATE_COEFFS = np.array([
    [0, 0, 0, 0], [0, 0, 0, 1], [0, 1, 0, -1], [0, 1, 0, 0],
    [0, 0, 1, -1], [0, 0, 1, 0], [0, 1, 1, -2], [0, 1, 1, -1],
    [1, -1, -1, 1], [1, -1, -1, 2], [1, 0, -1, 0], [1, 0, -1, 1],
    [1, -1, 0, 0], [1, -1, 0, 1], [1, 0, 0, -1], [1, 0, 0, 0],
], dtype=np.float32)  # [16, 4]

_CACHE = {}


def _build_nc(n_reps=1):
    import concourse.bacc as bacc
    import concourse.mybir as mybir
    from concourse.tile import TileContext

    f32 = mybir.dt.float32
    i16 = mybir.dt.int16
    Alu = mybir.AluOpType

    nc = bacc.Bacc("TRN2", target_bir_lowering=False, debug=False,
                   num_devices=N_CORES)
    x = nc.dram_tensor("x", [P, IN_DIM, N_BB], f32,
                       kind="ExternalInput").ap()
    idxw = nc.dram_tensor("idxw", [P, OUT_DIM // 8], i16,
                          kind="ExternalInput").ap()
    cbt = nc.dram_tensor("cbt", [P, 4, OUT_DIM], f32,
                         kind="ExternalInput").ap()
    y = nc.dram_tensor("y", [ROWS, OUT_DIM], f32, kind="ExternalOutput").ap()

    y_t = y.rearrange("(bb p) m -> bb p m", p=P)      # [4, 128, 8192]
    icols = OCHUNK // 16  # idx columns per chunk (64)

    with TileContext(nc) as tc:
        with tc.tile_pool(name="xr", bufs=1) as xpool, \
             tc.tile_pool(name="const", bufs=1) as cpool:
            xa = xpool.tile([P, IN_DIM, N_BB], f32, tag="xa")
            nc.sync.dma_start(out=xa[:], in_=x)
            idx_sb = cpool.tile([P, OUT_DIM // 8], i16, tag="idx")
            nc.sync.dma_start(out=idx_sb[:], in_=idxw)

            for rep in range(n_reps):
                with tc.tile_pool(name=f"co{rep}", bufs=2) as copool, \
                     tc.tile_pool(name=f"ab{rep}", bufs=2) as abpool, \
                     tc.tile_pool(name=f"qr{rep}", bufs=2) as qrpool:
                    for c in range(N_CHUNK):
                        cc = copool.tile([P, 4, OCHUNK], f32, tag="cc")
                        nc.sync.dma_start(
                            out=cc[:],
                            in_=cbt[:, :, c * OCHUNK:(c + 1) * OCHUNK])
                        ia = idx_sb[:, c * icols:(c + 1) * icols]
                        ib = idx_sb[:, OUT_DIM // 16 + c * icols:
                                    OUT_DIM // 16 + (c + 1) * icols]
                        c0 = cc[:, 0, :]
                        ca = cc[:, 1, :]
                        cb = cc[:, 2, :]
                        cab = cc[:, 3, :]
                        ga = abpool.tile([P, OCHUNK, N_BB], f32, tag="a")
                        nc.gpsimd.ap_gather(
                            out_ap=ga[:], in_ap=xa[:], idxs_ap=ia,
                            channels=P, num_elems=IN_DIM, d=N_BB,
                            num_idxs=OCHUNK)
                        gb = abpool.tile([P, OCHUNK, N_BB], f32, tag="b")
                        nc.gpsimd.ap_gather(
                            out_ap=gb[:], in_ap=xa[:], idxs_ap=ib,
                            channels=P, num_elems=IN_DIM, d=N_BB,
                            num_idxs=OCHUNK)
                        for bb in range(N_BB):
                            a = ga[:, :, bb]
                            b = gb[:, :, bb]
                            q = qrpool.tile([P, OCHUNK], f32, tag="q")
                            r = qrpool.tile([P, OCHUNK], f32, tag="r")
                            # q = (a*cab + cb) * b
                            nc.vector.tensor_mul(q[:], a, cab)
                            nc.vector.tensor_add(q[:], q[:], cb)
                            nc.vector.tensor_mul(q[:], q[:], b)
                            # r = a*ca + c0
                            nc.vector.tensor_mul(r[:], a, ca)
                            nc.vector.tensor_add(r[:], r[:], c0)
                            # out = q + r
                            nc.vector.tensor_add(q[:], q[:], r[:])
                            nc.sync.dma_start(
                                out=y_t[bb][:, c * OCHUNK:(c + 1) * OCHUNK],
                                in_=q[:])
    nc.compile()
    return nc


def _prep_host(x, weights, idx_a, idx_b):
    x = np.asarray(x, dtype=np.float32)
    w = np.asarray(weights, dtype=np.float32)
    e = np.exp(w - w.max(axis=1, keepdims=True))
    sm = e / e.sum(axis=1, keepdims=True)
    coeffs = (sm @ GATE_COEFFS).astype(np.float32)          # [8192, 4]
    cbt = np.ascontiguousarray(
        np.broadcast_to(coeffs.T[None, :, :], (P, 4, OUT_DIM))
    ).astype(np.float32)                                     # [128, 4, 8192]
    ia = np.asarray(idx_a).astype(np.int16)
    ib = np.asarray(idx_b).astype(np.int16)

    def wrap(seq):  # j = s*16 + p16 -> [16, len/16] -> tile to 128 partitions
        m = seq.reshape(len(seq) // 16, 16).T
        return np.tile(m, (P // 16, 1))

    idxw = np.ascontiguousarray(
        np.concatenate([wrap(ia), wrap(ib)], axis=1))        # [128, 1024]
    xi = []
    for c in range(N_CORES):
        sh = x[c * ROWS:(c + 1) * ROWS]          # [512, 8192]
        xi.append(np.ascontiguousarray(
            sh.reshape(N_BB, P, IN_DIM).transpose(1, 2, 0)))  # [128,8192,4]
    return xi, idxw, cbt


def _in_maps(x, weights, idx_a, idx_b):
    xi, idxw, cbt = _prep_host(x, weights, idx_a, idx_b)
    return [{"x": xi[c], "idxw": idxw, "cbt": cbt}
            for c in range(N_CORES)]


def kernel(x, weights, idx_a, idx_b):
    from concourse.bass_utils import run_bass_kernel_spmd

    in_maps = _in_maps(x, weights, idx_a, idx_b)
    if "nc" not in _CACHE:
        _CACHE["nc"] = _build_nc()
    nc = _CACHE["nc"]
    res = run_bass_kernel_spmd(nc, in_maps, list(range(N_CORES)))
    out = np.concatenate([res.results[c]["y"] for c in range(N_CORES)], axis=0)
    return out.astype(np.float32)
